# revision 12
# baseline (speedup 1.0000x reference)
"""Multi-head causal attention (B=2, S=2048, DIM=2048, H=16, HD=128) with RoPE,
distributed over 8 Trainium2 NeuronCores.

Sharding: data-parallel over batch (2) x tensor-parallel over head groups (4):
core = b*4 + g handles batch b, heads [4g, 4g+4). Each core computes
Q/K/V projections for its head group (bf16 matmuls, fp32 psum), applies RoPE,
runs causal attention in d-major layouts (no on-device transposes), applies
the output-projection rows for its heads, and returns a partial (S, DIM)
output in bf16. Host sums the 4 partials per batch (row-parallel wo).

v2 structure (vs v1):
  - HAM warmup: a burst of dummy matmuls at kernel start so the PE clock
    is at 2.4 GHz when real data arrives (v1 ran cold until ~39us).
  - Dual-lane DMA issue (sync + scalar HWDGE queues) to cut the input head.
  - Softmax row-sums come from a gpsimd partition_all_reduce over a
    DVE-accumulated P tile (fp16), replacing v1's ones-column matmul chain
    (~55us of PE time) and its DMA-transpose reciprocal round trip.
  - exp is batched over jt-pairs: one ACT instruction per [128,1024] psum
    pair-tile (2 banks), amortizing the ~293ns ACT fixed overhead.
  - Phases are interleaved A(s0) B(i0) A(s1) B(i1) ... C so scalar/DVE
    attention work overlaps projection matmuls.
"""

import numpy as np
import ml_dtypes

import concourse.bacc as bacc
import concourse.mybir as mybir
import concourse.tile as tile
from concourse import bass_isa
from concourse.bass_utils import run_bass_kernel_spmd

B, S, DIM, H, HD = 2, 2048, 2048, 16, 128
NCORES = 8
GROUPS = 4               # head groups (tensor-parallel)
HPC = H // GROUPS        # 4 heads per core
GD = HPC * HD            # 512 dims per group
NKT = DIM // 128         # 16 contraction tiles
NSB = S // 512           # 4 s blocks
NIB = S // 512           # 4 i blocks
F32 = mybir.dt.float32
BF16 = mybir.dt.bfloat16
FP16 = mybir.dt.float16
BF = ml_dtypes.bfloat16
NEG = -1e9
NWARM = 72               # HAM warmup matmuls

_CACHE = {}


def _build():
    nc = bacc.Bacc("TRN2", target_bir_lowering=False, debug=False,
                   num_devices=NCORES, dynamic_dma_scratch_size=4096)
    xT = nc.dram_tensor("xT", [DIM, S], BF16, kind="ExternalInput").ap()
    wq = nc.dram_tensor("wq", [DIM, GD], BF16, kind="ExternalInput").ap()
    wk = nc.dram_tensor("wk", [DIM, GD], BF16, kind="ExternalInput").ap()
    wv = nc.dram_tensor("wv", [DIM, GD], BF16, kind="ExternalInput").ap()
    wo = nc.dram_tensor("wo", [GD, DIM], BF16, kind="ExternalInput").ap()
    ropeC = nc.dram_tensor("ropeC", [HD, S], BF16, kind="ExternalInput").ap()
    ropeS = nc.dram_tensor("ropeS", [HD, S], BF16, kind="ExternalInput").ap()
    tri = nc.dram_tensor("tri", [128, 128], F32, kind="ExternalInput").ap()
    pmat = nc.dram_tensor("pmat", [128, 128], BF16, kind="ExternalInput").ap()
    out = nc.dram_tensor("out", [S, DIM], BF16, kind="ExternalOutput").ap()
    wscr = nc.dram_tensor("wscr", [128, 128], F32, kind="Internal").ap()

    with tile.TileContext(nc) as tc:
        with (
            tc.tile_pool(name="wpool", bufs=3 * NKT) as wpool,
            tc.tile_pool(name="xpool", bufs=16) as xpool,
            tc.tile_pool(name="qkpool", bufs=2 * HPC) as qkpool,
            tc.tile_pool(name="vpool", bufs=S // 128) as vpool,
            tc.tile_pool(name="otpool", bufs=HPC * NIB) as otpool,
            tc.tile_pool(name="wopool", bufs=2 * HPC) as wopool,
            tc.tile_pool(name="cpool", bufs=1) as cpool,
            tc.tile_pool(name="stage", bufs=3) as stage,
            tc.tile_pool(name="tpool", bufs=3) as tpool,
            tc.tile_pool(name="ptpool", bufs=4) as ptpool,
            tc.tile_pool(name="accpool", bufs=3) as accpool,
            tc.tile_pool(name="pspool", bufs=2) as pspool,
            tc.tile_pool(name="lpool", bufs=2) as lpool,
            tc.tile_pool(name="rlpool", bufs=2) as rlpool,
            tc.tile_pool(name="copool", bufs=8) as copool,
            tc.tile_pool(name="ps_mm", bufs=2, space="PSUM") as ps_mm,
            tc.tile_pool(name="ps_st", bufs=2, space="PSUM") as ps_st,
            tc.tile_pool(name="ps_o", bufs=2, space="PSUM") as ps_o,
        ):
            # ---- input DMAs: two issue lanes (sync + scalar HWDGE).
            # scalar lane: consts + odd wq/xt0 (head-critical, small so the
            # scalar queue frees up before phase-A copies need it).
            # sync lane: everything else, in consumption order.
            ropeC_t = cpool.tile([HD, S], BF16, tag="ropeC")
            nc.scalar.dma_start(ropeC_t[:], ropeC[:, :])
            ropeS_t = cpool.tile([HD, S], BF16, tag="ropeS")
            nc.scalar.dma_start(ropeS_t[:], ropeS[:, :])
            tri_t = cpool.tile([128, 128], F32, tag="tri")
            nc.scalar.dma_start(tri_t[:], tri[:, :])
            pmat_t = cpool.tile([128, 128], BF16, tag="pmat")
            nc.scalar.dma_start(pmat_t[:], pmat[:, :])

            wq_t, wk_t, wv_t = [], [], []
            xt_t = {}
            for kt in range(NKT):
                sl = slice(kt * 128, (kt + 1) * 128)
                eng = nc.sync if kt % 2 == 0 else nc.scalar
                t = wpool.tile([128, GD], BF16, tag="w", name=f"wq{kt}")
                eng.dma_start(t[:], wq[sl, :]); wq_t.append(t)
                eng2 = nc.scalar if kt % 2 == 0 else nc.sync
                t = xpool.tile([128, 512], BF16, tag="xt", name=f"xt0_{kt}")
                eng2.dma_start(t[:], xT[sl, 0:512]); xt_t[(0, kt)] = t
            for kt in range(NKT):
                sl = slice(kt * 128, (kt + 1) * 128)
                t = wpool.tile([128, GD], BF16, tag="w", name=f"wk{kt}")
                nc.sync.dma_start(t[:], wk[sl, :]); wk_t.append(t)
            for kt in range(NKT):
                sl = slice(kt * 128, (kt + 1) * 128)
                t = wpool.tile([128, GD], BF16, tag="w", name=f"wv{kt}")
                nc.sync.dma_start(t[:], wv[sl, :]); wv_t.append(t)
            for kt in range(NKT):
                sl = slice(kt * 128, (kt + 1) * 128)
                t = xpool.tile([128, 512], BF16, tag="xt", name=f"xt1_{kt}")
                nc.sync.dma_start(t[:], xT[sl, 512:1024]); xt_t[(1, kt)] = t
            for sb in (2, 3):
                for kt in range(NKT):
                    sl = slice(kt * 128, (kt + 1) * 128)
                    t = xpool.tile([128, 512], BF16, tag="xt",
                                   name=f"xt{sb}_{kt}")
                    nc.sync.dma_start(t[:], xT[sl, sb * 512:(sb + 1) * 512])
                    xt_t[(sb, kt)] = t

            # ---- HAM warmup: dummy matmul burst so the PE clock gate opens
            # before real data lands. Result is drained to a scratch DRAM
            # tensor so every tile has a reader.
            wps = ps_mm.tile([128, 128], F32, tag="mm", name="warmps")
            for i in range(NWARM):
                nc.tensor.matmul(wps[:], pmat_t[:], pmat_t[:],
                                 start=(i == 0), stop=(i == NWARM - 1))
            wdr = stage.tile([128, 128], F32, tag="wdr")
            nc.vector.tensor_copy(wdr[:], wps[:])
            nc.sync.dma_start(wscr[:, :], wdr[:])

            # persistent activations
            qt_t = [qkpool.tile([128, S], BF16, tag="qt", name=f"qt{h}")
                    for h in range(HPC)]
            kt_t = [qkpool.tile([128, S], BF16, tag="kt", name=f"ktt{h}")
                    for h in range(HPC)]
            v_t = [vpool.tile([128, GD], FP16, tag="v", name=f"v{st}")
                   for st in range(S // 128)]
            ot_t = {}
            for h in range(HPC):
                for ib in range(NIB):
                    ot_t[(h, ib)] = otpool.tile([128, 512], BF16, tag="ot",
                                                name=f"ot{h}_{ib}")

            # ---- phase A block: projections + rope for one 512-col s block.
            # The rope swap matmul for chain n is deferred until after chain
            # n+1's matmuls so the PE never waits on the psum->sbuf copy.
            rope_pend = [None]

            def _flush_rope():
                if rope_pend[0] is None:
                    return
                raw, t1, dst, h, s0 = rope_pend[0]
                rope_pend[0] = None
                sw = ps_mm.tile([128, 512], F32, tag="mm")
                nc.tensor.matmul(sw[:], pmat_t[:], raw[:],
                                 start=True, stop=True)
                t2 = tpool.tile([128, 512], BF16, tag="t2")
                nc.vector.tensor_mul(t2[:], sw[:], ropeS_t[:, s0:s0 + 512])
                nc.vector.tensor_add(dst[h][:, s0:s0 + 512], t1[:], t2[:])

            def emit_A(sb):
                s0 = sb * 512
                xt = [xt_t[(sb, kt)] for kt in range(NKT)]
                for w_t, dst in ((wq_t, qt_t), (wk_t, kt_t)):
                    for h in range(HPC):
                        pmm = ps_mm.tile([128, 512], F32, tag="mm")
                        for kt in range(NKT):
                            nc.tensor.matmul(
                                pmm[:],
                                w_t[kt][:, h * 128:(h + 1) * 128],
                                xt[kt][:],
                                start=(kt == 0), stop=(kt == NKT - 1),
                            )
                        raw = stage.tile([128, 512], BF16, tag="raw")
                        nc.scalar.copy(raw[:], pmm[:])
                        _flush_rope()
                        t1 = tpool.tile([128, 512], BF16, tag="t1")
                        nc.vector.tensor_mul(t1[:], raw[:],
                                             ropeC_t[:, s0:s0 + 512])
                        rope_pend[0] = (raw, t1, dst, h, s0)
                for st in range(4):
                    pmm = ps_mm.tile([128, 512], F32, tag="mm")
                    for kt in range(NKT):
                        nc.tensor.matmul(
                            pmm[:],
                            xt[kt][:, st * 128:(st + 1) * 128],
                            wv_t[kt][:],
                            start=(kt == 0), stop=(kt == NKT - 1),
                        )
                    if st == 0:
                        _flush_rope()
                    nc.scalar.copy(v_t[sb * 4 + st][:], pmm[:])

            # ---- phase B: attention for one 512-row i block, two heads at a
            # time (keeps PE fed while ACT runs exp of the other head).
            def emit_B_headpair(ib, heads):
                i0 = ib * 512
                njt = 4 * ib + 4
                npairs = njt // 2
                o_ps = {h: ps_o.tile([128, 512], F32, tag="o", name=f"o{ib}_{h}") for h in heads}
                acc = {}
                pts = {}
                for p in range(npairs):
                    jta, jtb = 2 * p, 2 * p + 1
                    va = max(0, jta * 128 - i0)
                    vb = max(0, jtb * 128 - i0)
                    st2s = {}
                    # full-width score matmuls: the masked cols get -1e9
                    # below (the overlap also orders DVE masking after the
                    # PE write -- disjoint writes to one psum bank are a
                    # fatal hardware conflict).
                    for h in heads:
                        st2 = ps_st.tile([128, 1024], F32, tag="st")
                        nc.tensor.matmul(
                            st2[:, 0:512],
                            kt_t[h][:, jta * 128:(jta + 1) * 128],
                            qt_t[h][:, i0:i0 + 512],
                            start=True, stop=True,
                        )
                        nc.tensor.matmul(
                            st2[:, 512:1024],
                            kt_t[h][:, jtb * 128:(jtb + 1) * 128],
                            qt_t[h][:, i0:i0 + 512],
                            start=True, stop=True,
                        )
                        st2s[h] = st2
                    # mask: diagonal blocks get tri; garbage cols (below
                    # voff) get -1e9 so exp() lands exactly 0 there.
                    if jtb * 128 >= i0 or va > 0 or vb > 0:
                        for h in heads:
                            st2 = st2s[h]
                            if va > 0:
                                nc.vector.memset(st2[:, 0:va], NEG)
                            if jta * 128 >= i0:
                                nc.vector.tensor_add(
                                    st2[:, va:va + 128],
                                    st2[:, va:va + 128], tri_t[:])
                            if vb > 0:
                                nc.vector.memset(st2[:, 512:512 + vb], NEG)
                            if jtb * 128 >= i0:
                                nc.vector.tensor_add(
                                    st2[:, 512 + vb:512 + vb + 128],
                                    st2[:, 512 + vb:512 + vb + 128], tri_t[:])
                    for h in heads:
                        pt = ptpool.tile([128, 1024], FP16, tag="pt")
                        nc.scalar.activation(
                            pt[:], st2s[h][:],
                            mybir.ActivationFunctionType.Exp)
                        pts[(h, p)] = pt
                    with nc.allow_low_precision(reason="fp16 softmax-denominator accumulate; l has ~1e4 dynamic range"):
                        for h in heads:
                            pt = pts[(h, p)]
                            a = accpool.tile([128, 1024], FP16, tag="acc")
                            if p == 0:
                                nc.vector.tensor_copy(a[:], pt[:])
                            else:
                                nc.vector.tensor_add(a[:], acc[h][:], pt[:])
                            acc[h] = a
                    if p > 0:
                        for h in heads:
                            ptp = pts.pop((h, p - 1))
                            _emit_o(ib, h, p - 1, ptp, o_ps[h])
                for h in heads:
                    ptp = pts.pop((h, npairs - 1))
                    _emit_o(ib, h, npairs - 1, ptp, o_ps[h])
                # normalization: l = colsum(P) via partition all-reduce
                for h in heads:
                    psum_t = pspool.tile([128, 512], FP16, tag="psum")
                    with nc.allow_low_precision(reason="fp16 softmax-denominator fold"):
                        nc.vector.tensor_add(psum_t[:], acc[h][:, 0:512],
                                             acc[h][:, 512:1024])
                    lall = lpool.tile([128, 512], FP16, tag="lall")
                    nc.gpsimd.partition_all_reduce(
                        lall[:], psum_t[:], channels=128,
                        reduce_op=bass_isa.ReduceOp.add)
                    rl = rlpool.tile([128, 512], FP16, tag="rl")
                    with nc.allow_low_precision(reason="fp16 reciprocal of softmax denominator"):
                        nc.vector.reciprocal(rl[:], lall[:])
                    nc.vector.tensor_mul(ot_t[(h, ib)][:], o_ps[h][:], rl[:])

            def _emit_o(ib, h, p, pt, o_ps):
                i0 = ib * 512
                njt = 4 * ib + 4
                for half, jt in ((0, 2 * p), (1, 2 * p + 1)):
                    v = max(0, jt * 128 - i0)
                    nc.tensor.matmul(
                        o_ps[:, v:512],
                        v_t[jt][:, h * 128:(h + 1) * 128],
                        pt[:, half * 512 + v:half * 512 + 512],
                        start=(jt == 0), stop=(jt == njt - 1),
                    )

            def emit_B(ib):
                emit_B_headpair(ib, (0, 1))
                emit_B_headpair(ib, (2, 3))

            # ---- phase C: output projection, eb-major so the wo tiles can
            # be streamed in (halves their SBUF footprint) and reused across
            # all 16 s tiles.
            wo_t = {}

            def _wo_dma(eb):
                for h in range(HPC):
                    t = wopool.tile([128, 512], BF16, tag="wo",
                                    name=f"wo{eb}_{h}")
                    nc.sync.dma_start(
                        t[:],
                        wo[h * 128:(h + 1) * 128, eb * 512:(eb + 1) * 512])
                    wo_t[(h, eb)] = t

            def emit_C_all():
                _wo_dma(0)
                _wo_dma(1)
                for eb in range(4):
                    if eb + 2 < 4:
                        _wo_dma(eb + 2)
                    for stile in range(S // 128):
                        ib, soff = stile // 4, (stile % 4) * 128
                        pmm = ps_mm.tile([128, 512], F32, tag="mm")
                        for h in range(HPC):
                            nc.tensor.matmul(
                                pmm[:],
                                ot_t[(h, ib)][:, soff:soff + 128],
                                wo_t[(h, eb)][:],
                                start=(h == 0), stop=(h == HPC - 1),
                            )
                        co = copool.tile([128, 512], BF16, tag="co")
                        if stile % 2 == 0:
                            nc.scalar.copy(co[:], pmm[:])
                        else:
                            nc.vector.tensor_copy(co[:], pmm[:])
                        nc.sync.dma_start(
                            out[stile * 128:(stile + 1) * 128,
                                eb * 512:(eb + 1) * 512],
                            co[:],
                        )

            emit_A(0)
            for k in range(NSB):
                emit_B(k)
                if k + 1 < NSB:
                    emit_A(k + 1)
            emit_C_all()

    nc.compile()
    return nc


def _host_inputs(x, freqs_cos, freqs_sin, wq, wk, wv, wo):
    """Build the 8 per-core input maps (host-side sharding + layout prep)."""
    scale = 1.0 / np.sqrt(HD)
    # rope tables, d-major duplicated/interleaved: C[d,s]=cos[s,d//2];
    # S[2j,s]=-sin[s,j]; S[2j+1,s]=+sin[s,j]
    c = np.asarray(freqs_cos, dtype=np.float32)      # (S, HD/2)
    s = np.asarray(freqs_sin, dtype=np.float32)
    ropeC = np.repeat(c.T, 2, axis=0)                # (HD, S)
    ropeS = np.empty((HD, S), dtype=np.float32)
    ropeS[0::2] = -s.T
    ropeS[1::2] = s.T
    ropeC = ropeC.astype(BF)
    ropeS = ropeS.astype(BF)

    tri = np.where(
        np.arange(128)[:, None] <= np.arange(128)[None, :], 0.0, NEG
    ).astype(np.float32)
    pmat = np.zeros((128, 128), dtype=np.float32)
    idx = np.arange(128)
    pmat[idx, idx ^ 1] = 1.0
    pmat = pmat.astype(BF)

    xT = [np.ascontiguousarray(np.asarray(x[b]).T).astype(BF) for b in range(B)]
    wq = np.asarray(wq, dtype=np.float32)
    wk = np.asarray(wk, dtype=np.float32)
    wv = np.asarray(wv, dtype=np.float32)
    wo = np.asarray(wo, dtype=np.float32)

    in_maps = []
    for core in range(NCORES):
        b, g = core // GROUPS, core % GROUPS
        cols = slice(g * GD, (g + 1) * GD)
        in_maps.append({
            "xT": xT[b],
            "wq": np.ascontiguousarray(wq[:, cols] * scale).astype(BF),
            "wk": np.ascontiguousarray(wk[:, cols]).astype(BF),
            "wv": np.ascontiguousarray(wv[:, cols]).astype(BF),
            "wo": np.ascontiguousarray(wo[cols, :]).astype(BF),
            "ropeC": ropeC,
            "ropeS": ropeS,
            "tri": tri,
            "pmat": pmat,
        })
    return in_maps


def _get_nc():
    if "nc" not in _CACHE:
        _CACHE["nc"] = _build()
    return _CACHE["nc"]


def run(inputs, trace=False, tmpdir=None):
    """Run on hardware; returns (full_output, BassKernelResults)."""
    nc = _get_nc()
    in_maps = _host_inputs(
        inputs["x"], inputs["freqs_cos"], inputs["freqs_sin"],
        inputs["wq"], inputs["wk"], inputs["wv"], inputs["wo"],
    )
    res = run_bass_kernel_spmd(
        nc, in_maps, core_ids=list(range(NCORES)), trace=trace, tmpdir=tmpdir
    )
    outs = [np.asarray(res.results[c]["out"], dtype=np.float32)
            for c in range(NCORES)]
    full = np.stack(
        [sum(outs[b * GROUPS + g] for g in range(GROUPS)) for b in range(B)],
        axis=0,
    )
    return full, res


def kernel(**inputs) -> np.ndarray:
    full, _ = run(inputs, trace=False)
    return full
